# revision 16
# baseline (speedup 1.0000x reference)
"""Trainium2 Bass kernel for a bipartite GNN message-passing layer.

Strategy (8 NeuronCores, data-parallel over batch B=16 -> 2 batches/core):
  Per phase (D->E then E->D):
    1. transform-first: tX = hX @ W_msg^T computed per 128-row tile from
       host-pre-transposed activations (hXT input), written interleaved for
       both local batches to a DRAM scratch [N_src, 256] so one gather row
       serves both batches.
    2. edges are sorted by destination on the host and padded per dst tile
       to multiples of 128; dma_gather (2 SWDGE queues, multi-packet) pulls
       the per-edge source rows into SBUF.
    3. scatter-add realized as one-hot matmuls: O[e, dst] = (dl[e]==dst)
       built on-device in one tensor_tensor per dst tile (broadcast APs);
       PSUM accumulates agg[dst, 256] over the tile's edge chunks.
    4. epilogue per dst tile: agg*scale (ACT) + self-matmul PSUM add, relu,
       residual add, layernorm via sum/sumsq (ACT Square+accum) and
       rstd = exp(-0.5*ln(var+eps)) on ACT; normalize via ACT Identity
       with per-partition scale/bias. Writes the output and (phase 1) the
       next phase's transformed+interleaved scratch.

Engine budget notes (from NTFF profiling): avoid tensor_scalar with AP
scalars (~1us/op vs ~0.26us tensor_tensor), avoid DVE accum_out
(READ_ACCUMULATOR ~0.9us), avoid nc.vector.reciprocal (~1.9us), avoid two
PSUM operands in one DVE op (verifier rejects), dma_gather needs
single_packet=False above ~1k indices.
"""

import math
import os
import sys

import numpy as np

sys.path.insert(0, "/opt/trn_rl_repo")

import concourse.bacc as bacc
import concourse.bass as bass
import concourse.tile as tile
from concourse import mybir
from concourse.bass_utils import run_bass_kernel_spmd
from concourse.masks import make_identity

P = 128
H = 128
EPS = 1e-5
F32 = mybir.dt.float32
I32 = mybir.dt.int32
I16 = mybir.dt.int16
AF = mybir.ActivationFunctionType
OP = mybir.AluOpType

N_CORES = 8
MAXCH = 32  # max gather chunks (of 128 edges) per dma_gather group


# ----------------------------------------------------------------- host prep

def _prep_edges(edge_src, edge_dst, n_dst):
    """Sort edges by dst; per dst tile pad the edge list to a multiple of 128.
    Tiles with zero edges get zero chunks (skipped on device)."""
    order = np.argsort(edge_dst, kind="stable")
    src_s = edge_src[order]
    dst_s = edge_dst[order]
    n_tiles = n_dst // P
    bounds = np.searchsorted(dst_s, np.arange(n_tiles + 1) * P)
    g_src, g_dl, cpt = [], [], []
    for t in range(n_tiles):
        lo, hi = bounds[t], bounds[t + 1]
        n = hi - lo
        nch = (n + P - 1) // P
        pad = nch * P - n
        if n > 0:
            g_src.append(src_s[lo:hi])
            g_dl.append(dst_s[lo:hi] - t * P)
            if pad:
                g_src.append(np.zeros(pad, np.int64))
                g_dl.append(np.full(pad, -1, np.int64))
        cpt.append(nch)
    gather_src = np.concatenate(g_src).astype(np.int64) if g_src else np.zeros(0, np.int64)
    dl = np.concatenate(g_dl).astype(np.int32) if g_dl else np.zeros(0, np.int32)
    return gather_src, dl, np.array(cpt, np.int64)


def _wrap_idx16(idx):
    """[n] -> [128, n//16] int16, wrapped in 16 partitions, replicated x8."""
    n = len(idx)
    assert n % 16 == 0
    w = idx.astype(np.int16).reshape(n // 16, 16).T  # [16, n//16]
    return np.tile(w, (8, 1)).copy()  # [128, n//16]


def _col_mat(v, n_tiles):
    """[n_tiles*128] -> [128, n_tiles] where col t = v[t*128:(t+1)*128]."""
    return np.ascontiguousarray(v.reshape(n_tiles, P).T)


def _col_mat_chunks(dl):
    """padded dl [padE] -> [128, n_chunks] f32 (col c = chunk c's 128 values)."""
    if len(dl) == 0:
        return np.zeros((P, 1), np.float32)
    n_ch = len(dl) // P
    return np.ascontiguousarray(dl.reshape(n_ch, P).T.astype(np.float32))


def _make_groups(cpt):
    """Greedy-pack active dst tiles into gather groups of <= MAXCH chunks."""
    groups = []
    cur, cur_ch = [], 0
    for t, c in enumerate(cpt):
        if c == 0:
            continue
        if cur and cur_ch + c > MAXCH:
            groups.append((cur, cur_ch))
            cur, cur_ch = [], 0
        cur.append(t)
        cur_ch += int(c)
    if cur:
        groups.append((cur, cur_ch))
    return groups


# ------------------------------------------------------------ program build

class PhaseCfg:
    def __init__(self, name, n_src, n_dst, cpt, padE, use_sigmoid):
        self.name = name
        self.n_src = n_src
        self.n_dst = n_dst
        self.cpt = cpt
        self.padE = padE
        self.use_sigmoid = use_sigmoid
        self.groups = _make_groups(cpt)
        self.n_dst_tiles = n_dst // P
        self.n_chunks = int(cpt.sum())


def build_program(Bc, N_d, N_e, ph1, ph2, n_src_tiles_t2, trivial1, trivial2,
                  num_devices=N_CORES):
    """Emit the full SPMD Bass program."""
    nc = bacc.Bacc("TRN2", target_bir_lowering=False, debug=False,
                   enable_asserts=False, num_devices=num_devices,
                   num_swdge_queues=2, dynamic_dma_scratch_size=32768)
    BH = Bc * H

    # ---- DRAM tensors
    hD = nc.dram_tensor("hD", [Bc, N_d, H], F32, kind="ExternalInput").ap()
    hE = nc.dram_tensor("hE", [Bc, N_e, H], F32, kind="ExternalInput").ap()
    hDT = nc.dram_tensor("hDT", [Bc, H, N_d], F32, kind="ExternalInput").ap()
    hET = nc.dram_tensor("hET", [Bc, H, N_e], F32, kind="ExternalInput").ap()
    WmsgT1 = nc.dram_tensor("WmsgT1", [H, H], F32, kind="ExternalInput").ap()
    WselfT1 = nc.dram_tensor("WselfT1", [H, H], F32, kind="ExternalInput").ap()
    WmsgT2 = nc.dram_tensor("WmsgT2", [H, H], F32, kind="ExternalInput").ap()
    WselfT2 = nc.dram_tensor("WselfT2", [H, H], F32, kind="ExternalInput").ap()
    ew_mat = nc.dram_tensor("ew_mat", [P, N_e // P], F32, kind="ExternalInput").ap()
    cnt1_mat = nc.dram_tensor("cnt1_mat", [P, N_e // P], F32, kind="ExternalInput").ap()
    cnt2_mat = nc.dram_tensor("cnt2_mat", [P, N_d // P], F32, kind="ExternalInput").ap()
    idx1 = nc.dram_tensor("idx1", [P, ph1.padE // 16], I16, kind="ExternalInput").ap()
    dl1 = nc.dram_tensor("dl1", [P, max(ph1.n_chunks, 1)], F32, kind="ExternalInput").ap()
    idx2 = nc.dram_tensor("idx2", [P, ph2.padE // 16], I16, kind="ExternalInput").ap()
    dl2 = nc.dram_tensor("dl2", [P, max(ph2.n_chunks, 1)], F32, kind="ExternalInput").ap()
    aff1 = nc.dram_tensor("aff1", [3, H], F32, kind="ExternalInput").ap()
    aff2 = nc.dram_tensor("aff2", [3, H], F32, kind="ExternalInput").ap()

    hE_new = nc.dram_tensor("hE_new", [Bc, N_e, H], F32, kind="ExternalOutput").ap()
    hD_new = nc.dram_tensor("hD_new", [Bc, N_d, H], F32, kind="ExternalOutput").ap()

    tD = nc.dram_tensor("tD_scratch", [N_d, BH], F32, kind="Internal").ap()
    tE = nc.dram_tensor("tE_scratch", [n_src_tiles_t2 * P, BH], F32, kind="Internal").ap()

    with tile.TileContext(nc) as tc:
        import contextlib
        with contextlib.ExitStack() as ctx:
            # PSUM: every tile is padded to a full bank (8 banks total):
            # tr(2) + ps_agg(2) + ps_se(2) + ps_mm(2) = 8.
            const_tp = ctx.enter_context(tc.tile_pool(name="const", bufs=1))
            psum_tp = ctx.enter_context(tc.tile_pool(name="ps", bufs=2, space="PSUM"))
            tr_tp = ctx.enter_context(tc.tile_pool(name="tr", bufs=2, space="PSUM"))
            sb_tp = ctx.enter_context(tc.tile_pool(name="sb", bufs=3))
            g_tp = ctx.enter_context(tc.tile_pool(name="gath", bufs=2))

            ident = const_tp.tile([P, P], F32, tag="ident")
            make_identity(nc, ident[:])
            iota_i = const_tp.tile([P, P], I32, tag="iota_i")
            nc.gpsimd.iota(iota_i[:], pattern=[[1, P]], base=0, channel_multiplier=0)
            iota = const_tp.tile([P, P], F32, tag="iota")
            nc.vector.tensor_copy(iota[:], iota_i[:])
            # small per-partition constants
            eps_t = const_tp.tile([P, 1], F32, tag="eps")
            nc.gpsimd.memset(eps_t[:], EPS)
            invH_t = const_tp.tile([P, 1], F32, tag="invH")
            nc.gpsimd.memset(invH_t[:], 1.0 / H)
            neginvH_t = const_tp.tile([P, 1], F32, tag="neginvH")
            nc.gpsimd.memset(neginvH_t[:], -1.0 / H)
            neghalf_t = const_tp.tile([P, 1], F32, tag="neghalf")
            nc.gpsimd.memset(neghalf_t[:], -0.5)

            def load_const(ap_dram, shape, tag):
                t = const_tp.tile(shape, F32, tag=tag)
                nc.sync.dma_start(t[:], ap_dram)
                return t

            w_msg1 = load_const(WmsgT1[:, :], [P, H], "w0")
            w_self1 = load_const(WselfT1[:, :], [P, H], "w1")
            w_msg2 = load_const(WmsgT2[:, :], [P, H], "w2")
            w_self2 = load_const(WselfT2[:, :], [P, H], "w3")

            # scale1 = sigmoid(ew) / max(cnt1, 1);  scale2 = 1 / max(cnt2, 1)
            nt1, nt2 = N_e // P, N_d // P
            sc_a = const_tp.tile([P, nt1], F32, tag="sc_a")
            sc_b = const_tp.tile([P, nt1], F32, tag="sc_b")
            sc_bs = const_tp.tile([P, nt1], F32, tag="sc_bs")
            scale1 = const_tp.tile([P, nt1], F32, tag="scale1")
            nc.sync.dma_start(sc_a[:], ew_mat[:, :])
            nc.sync.dma_start(sc_b[:], cnt1_mat[:, :])
            nc.scalar.activation(sc_a[:], sc_a[:], AF.Sigmoid)
            nc.vector.tensor_scalar_max(sc_b[:], sc_b[:], 1.0)
            nc.vector.reciprocal_approx_accurate(sc_b[:], sc_b[:], scratch=sc_bs[:])
            nc.vector.tensor_tensor(out=scale1[:], in0=sc_a[:], in1=sc_b[:], op=OP.mult)
            sc_c = const_tp.tile([P, nt2], F32, tag="sc_c")
            sc_cs = const_tp.tile([P, nt2], F32, tag="sc_cs")
            scale2 = const_tp.tile([P, nt2], F32, tag="scale2")
            nc.sync.dma_start(sc_c[:], cnt2_mat[:, :])
            nc.vector.tensor_scalar_max(sc_c[:], sc_c[:], 1.0)
            nc.vector.reciprocal_approx_accurate(scale2[:], sc_c[:], scratch=sc_cs[:])

            # index / dst-local tables
            idx1_sb = const_tp.tile([P, ph1.padE // 16], I16, tag="idx1")
            nc.sync.dma_start(idx1_sb[:], idx1[:, :])
            dl1_sb = const_tp.tile([P, max(ph1.n_chunks, 1)], F32, tag="dl1")
            nc.sync.dma_start(dl1_sb[:], dl1[:, :])
            idx2_sb = const_tp.tile([P, ph2.padE // 16], I16, tag="idx2")
            nc.sync.dma_start(idx2_sb[:], idx2[:, :])
            dl2_sb = const_tp.tile([P, max(ph2.n_chunks, 1)], F32, tag="dl2")
            nc.sync.dma_start(dl2_sb[:], dl2[:, :])

            # affine params (bias matmul trick + broadcast g/beta)
            def affine_setup(aff_dram, trivial, tag):
                if trivial:
                    return None
                e1 = const_tp.tile([P, P], F32, tag=f"e1{tag}")
                nc.gpsimd.memset(e1[:], 0.0)
                nc.gpsimd.memset(e1[0:1, :], 1.0)
                bpad = const_tp.tile([P, BH], F32, tag=f"bp{tag}")
                nc.gpsimd.memset(bpad[:], 0.0)
                for b in range(Bc):
                    nc.sync.dma_start(bpad[0:1, b * H:(b + 1) * H], aff_dram[0:1, :])
                g_mat = const_tp.tile([P, H], F32, tag=f"g{tag}")
                beta_mat = const_tp.tile([P, H], F32, tag=f"bt{tag}")
                one_row = const_tp.tile([1, P], F32, tag=f"one{tag}")
                nc.gpsimd.memset(one_row[:], 1.0)
                gb_row = const_tp.tile([1, 2 * H], F32, tag=f"gbr{tag}")
                nc.sync.dma_start(gb_row[0:1, 0:H], aff_dram[1:2, :])
                nc.sync.dma_start(gb_row[0:1, H:2 * H], aff_dram[2:3, :])
                gb_ps = tr_tp.tile([P, 2 * H], F32, tag="tr")
                nc.tensor.matmul(out=gb_ps[:], lhsT=one_row[0:1, 0:P],
                                 rhs=gb_row[0:1, :], start=True, stop=True)
                nc.vector.tensor_copy(g_mat[:], gb_ps[:, 0:H])
                nc.vector.tensor_copy(beta_mat[:], gb_ps[:, H:2 * H])
                return e1, bpad, g_mat, beta_mat

            aff1_t = affine_setup(aff1, trivial1, "1")
            aff2_t = affine_setup(aff2, trivial2, "2")

            # ---------------- tD pass: tD = hD @ Wmsg1^T (from hDT, no transposes)
            for i in range(N_d // P):
                rows = slice(i * P, (i + 1) * P)
                hdT = sb_tp.tile([P, BH], F32, tag="td_hT")
                for b in range(Bc):
                    nc.sync.dma_start(hdT[:, b * H:(b + 1) * H], hDT[b, :, rows])
                ps_td = psum_tp.tile([P, BH], F32, tag="ps_mm")
                for b in range(Bc):
                    nc.tensor.matmul(out=ps_td[:, b * H:(b + 1) * H],
                                     lhsT=hdT[:, b * H:(b + 1) * H],
                                     rhs=w_msg1[:], start=True, stop=True)
                stage = sb_tp.tile([P, BH], F32, tag="td_stage")
                nc.scalar.activation(stage[:], ps_td[:], AF.Copy)
                nc.sync.dma_start(tD[rows, :], stage[:])

            # ---------------- phases
            def emit_phase(ph, t_dram, self_dram, selfT_dram, out_dram, w_self,
                           scale_mat, idx_sb, dl_sb, aff_t, next_cfg, qbase):
                chunk_base = {}
                off = 0
                for t in range(ph.n_dst_tiles):
                    chunk_base[t] = off
                    off += int(ph.cpt[t])

                group_edge_off = []
                off_e = 0
                for (tids, nch) in ph.groups:
                    group_edge_off.append(off_e)
                    off_e += nch * P

                group_of_tile = {}
                for gi, (tids, nch) in enumerate(ph.groups):
                    for t in tids:
                        group_of_tile[t] = gi

                agg_ready = {}

                def do_group(gi):
                    tids, nch = ph.groups[gi]
                    g_tile = g_tp.tile([P, MAXCH, BH], F32, tag="G")
                    n_idx = nch * P
                    off16 = group_edge_off[gi] // 16
                    nc.gpsimd.dma_gather(
                        out_ap=g_tile[:, 0:nch, :],
                        in_ap=t_dram[:, :],
                        idxs_ap=idx_sb[:, off16:off16 + n_idx // 16],
                        num_idxs=n_idx,
                        num_idxs_reg=n_idx,
                        elem_size=BH,
                        single_packet=False,
                        queue_num=(qbase + gi) % 2,
                    )
                    lc = 0
                    for t in tids:
                        nch_t = int(ph.cpt[t])
                        cc0 = chunk_base[t]
                        # one-hot for ALL chunks of this tile in one DVE op
                        oh = sb_tp.tile([P, 9, P], F32, tag="onehot")
                        nc.vector.tensor_tensor(
                            out=oh[:, 0:nch_t, :],
                            in0=iota[:, None, :].to_broadcast([P, nch_t, P]),
                            in1=dl_sb[:, cc0:cc0 + nch_t, None].to_broadcast([P, nch_t, P]),
                            op=OP.is_equal)
                        ps_agg = psum_tp.tile([P, BH], F32, tag="ps_agg")
                        for c in range(nch_t):
                            nc.tensor.matmul(out=ps_agg[:], lhsT=oh[:, c, :],
                                             rhs=g_tile[:, lc + c, :],
                                             start=(c == 0), stop=(c == nch_t - 1))
                        lc += nch_t
                        agg_ready[t] = ps_agg

                emitted = set()
                for t in range(ph.n_dst_tiles):
                    gi = group_of_tile.get(t)
                    if gi is not None and gi not in emitted:
                        do_group(gi)
                        emitted.add(gi)

                    rows = slice(t * P, (t + 1) * P)
                    h_self = sb_tp.tile([P, BH], F32, tag="h_self")
                    heT = sb_tp.tile([P, BH], F32, tag="ep_hT")
                    for b in range(Bc):
                        nc.sync.dma_start(h_self[:, b * H:(b + 1) * H],
                                          self_dram[b, rows, :])
                        nc.sync.dma_start(heT[:, b * H:(b + 1) * H],
                                          selfT_dram[b, :, rows])
                    ps_se = psum_tp.tile([P, BH], F32, tag="ps_se")
                    if aff_t is not None:
                        e1, bpad, g_mat, beta_mat = aff_t
                        nc.tensor.matmul(out=ps_se[:], lhsT=e1[:], rhs=bpad[:],
                                         start=True, stop=False, skip_group_check=True)
                    for b in range(Bc):
                        nc.tensor.matmul(out=ps_se[:, b * H:(b + 1) * H],
                                         lhsT=heT[:, b * H:(b + 1) * H],
                                         rhs=w_self[:],
                                         start=(aff_t is None), stop=True,
                                         skip_group_check=True)

                    # v = relu(agg*scale + sE)
                    v_t = sb_tp.tile([P, BH], F32, tag="v")
                    ps_agg = agg_ready.pop(t, None)
                    if ps_agg is not None:
                        agg_s = sb_tp.tile([P, BH], F32, tag="agg_s")
                        nc.scalar.activation(agg_s[:], ps_agg[:], AF.Copy,
                                             scale=scale_mat[:, t:t + 1])
                        u_t = sb_tp.tile([P, BH], F32, tag="u")
                        nc.vector.tensor_tensor(out=u_t[:], in0=agg_s[:],
                                                in1=ps_se[:], op=OP.add)
                        nc.scalar.activation(v_t[:], u_t[:], AF.Relu)
                    else:
                        nc.scalar.activation(v_t[:], ps_se[:], AF.Relu)

                    # x = h_self + v ; s1 = rowsum(x) ; s2 = rowsum(x^2)
                    x_t = sb_tp.tile([P, BH], F32, tag="x")
                    nc.vector.tensor_tensor(out=x_t[:], in0=h_self[:],
                                            in1=v_t[:], op=OP.add)
                    stats = sb_tp.tile([P, 12], F32, tag="stats")
                    s1 = stats[:, 0:Bc]
                    s2 = stats[:, 2:2 + Bc]
                    nc.vector.tensor_reduce(
                        out=s1, in_=x_t[:].rearrange("p (b h) -> p b h", b=Bc),
                        axis=mybir.AxisListType.X, op=OP.add)
                    junk = sb_tp.tile([P, BH], F32, tag="junk")
                    for b in range(Bc):
                        bs = slice(b * H, (b + 1) * H)
                        nc.scalar.activation(junk[:, bs], x_t[:, bs], AF.Square,
                                             accum_out=stats[:, 2 + b:3 + b])
                    # tiny chain: VS = s2 - s1^2/H ; rstd = exp(-0.5*ln(VS/H+eps))
                    t1 = stats[:, 4:4 + Bc]
                    nc.vector.tensor_tensor(out=t1, in0=s1, in1=s1, op=OP.mult)
                    t2c = stats[:, 6:6 + Bc]
                    nc.vector.tensor_tensor(
                        out=t2c, in0=t1,
                        in1=invH_t[:, 0:1].to_broadcast([P, Bc]), op=OP.mult)
                    vs = stats[:, 8:8 + Bc]
                    nc.vector.tensor_tensor(out=vs, in0=s2, in1=t2c, op=OP.subtract)
                    lnv = sb_tp.tile([P, 6], F32, tag="lnv")
                    nc.scalar.activation(lnv[:, 0:Bc], vs, AF.Ln,
                                         scale=invH_t[:, 0:1], bias=eps_t[:, 0:1])
                    rstd = lnv[:, 2:2 + Bc]
                    nc.scalar.activation(rstd, lnv[:, 0:Bc], AF.Exp,
                                         scale=neghalf_t[:, 0:1])
                    negm = stats[:, 10:10 + Bc]
                    nc.vector.tensor_tensor(
                        out=negm, in0=s1,
                        in1=neginvH_t[:, 0:1].to_broadcast([P, Bc]), op=OP.mult)
                    negmr = lnv[:, 4:4 + Bc]
                    nc.vector.tensor_tensor(out=negmr, in0=negm, in1=rstd, op=OP.mult)

                    # y = x*rstd + negmr  [per batch; ACT Identity w/ scale+bias]
                    y_t = sb_tp.tile([P, BH], F32, tag="y")
                    for b in range(Bc):
                        bs = slice(b * H, (b + 1) * H)
                        nc.scalar.activation(y_t[:, bs], x_t[:, bs], AF.Identity,
                                             scale=rstd[:, b:b + 1],
                                             bias=negmr[:, b:b + 1])
                    if aff_t is not None:
                        e1, bpad, g_mat, beta_mat = aff_t
                        y3 = y_t[:].rearrange("p (b h) -> p b h", b=Bc)
                        nc.vector.tensor_tensor(
                            out=y3, in0=y3,
                            in1=g_mat[:, None, :].to_broadcast([P, Bc, H]), op=OP.mult)
                        nc.vector.tensor_tensor(
                            out=y3, in0=y3,
                            in1=beta_mat[:, None, :].to_broadcast([P, Bc, H]), op=OP.add)
                    for b in range(Bc):
                        nc.sync.dma_start(out_dram[b, rows, :],
                                          y_t[:, b * H:(b + 1) * H])

                    # fused next-phase transform: t2 = y @ Wmsg2^T
                    if next_cfg is not None:
                        w_next, t2_dram, n_src_tiles_next = next_cfg
                        if t < n_src_tiles_next:
                            ps_t2 = psum_tp.tile([P, BH], F32, tag="ps_mm")
                            for b in range(Bc):
                                ps_tr2 = tr_tp.tile([P, P], F32, tag="tr")
                                nc.tensor.transpose(out=ps_tr2[:],
                                                    in_=y_t[:, b * H:(b + 1) * H],
                                                    identity=ident[:])
                                yT = sb_tp.tile([P, P], F32, tag="t2_yT")
                                nc.scalar.activation(yT[:], ps_tr2[:], AF.Copy)
                                nc.tensor.matmul(out=ps_t2[:, b * H:(b + 1) * H],
                                                 lhsT=yT[:], rhs=w_next[:],
                                                 start=True, stop=True)
                            t2_stage = sb_tp.tile([P, BH], F32, tag="t2_stage")
                            nc.scalar.activation(t2_stage[:], ps_t2[:], AF.Copy)
                            nc.sync.dma_start(t2_dram[rows, :], t2_stage[:])

            emit_phase(ph1, tD, hE, hET, hE_new, w_self1, scale1, idx1_sb, dl1_sb,
                       aff1_t, (w_msg2, tE, n_src_tiles_t2), qbase=0)
            emit_phase(ph2, tE, hD, hDT, hD_new, w_self2, scale2, idx2_sb, dl2_sb,
                       aff2_t, None, qbase=1)

    nc.compile()
    return nc


# ----------------------------------------------------------------- kernel()

def _host_prep(hD, hE, edge_d2e, edge_e2d, error_weights):
    B, N_d, _ = hD.shape
    N_e = hE.shape[1]
    e1s, e1d = np.asarray(edge_d2e[0], np.int64), np.asarray(edge_d2e[1], np.int64)
    e2s, e2d = np.asarray(edge_e2d[0], np.int64), np.asarray(edge_e2d[1], np.int64)

    gs1, dl1, cpt1 = _prep_edges(e1s, e1d, N_e)
    gs2, dl2, cpt2 = _prep_edges(e2s, e2d, N_d)
    cnt1 = np.bincount(e1d, minlength=N_e).astype(np.float32)
    cnt2 = np.bincount(e2d, minlength=N_d).astype(np.float32)

    n_src_tiles_t2 = int(math.ceil((int(e2s.max()) + 1) / P)) if len(e2s) else 1

    prep = dict(
        idx1=_wrap_idx16(gs1), dl1=_col_mat_chunks(dl1),
        idx2=_wrap_idx16(gs2), dl2=_col_mat_chunks(dl2),
        ew_mat=_col_mat(np.asarray(error_weights, np.float32), N_e // P),
        cnt1_mat=_col_mat(cnt1, N_e // P),
        cnt2_mat=_col_mat(cnt2, N_d // P),
    )
    return prep, cpt1, cpt2, len(gs1), len(gs2), n_src_tiles_t2


def kernel(hD, hE, error_weights, W_d2e, W_e_self, b_e, g_e, beta_e,
           W_e2d, W_d_self, b_d, g_d, beta_d, edge_d2e, edge_e2d):
    hD = np.ascontiguousarray(np.asarray(hD, np.float32))
    hE = np.ascontiguousarray(np.asarray(hE, np.float32))
    B, N_d, _ = hD.shape
    N_e = hE.shape[1]
    assert B % N_CORES == 0
    Bc = B // N_CORES

    prep, cpt1, cpt2, padE1, padE2, n_src_tiles_t2 = _host_prep(
        hD, hE, edge_d2e, edge_e2d, error_weights)

    trivial1 = (np.all(np.asarray(b_e) == 0) and np.all(np.asarray(g_e) == 1)
                and np.all(np.asarray(beta_e) == 0))
    trivial2 = (np.all(np.asarray(b_d) == 0) and np.all(np.asarray(g_d) == 1)
                and np.all(np.asarray(beta_d) == 0))

    ph1 = PhaseCfg("d2e", N_d, N_e, cpt1, padE1, True)
    ph2 = PhaseCfg("e2d", n_src_tiles_t2 * P, N_d, cpt2, padE2, False)

    nc = build_program(Bc, N_d, N_e, ph1, ph2, n_src_tiles_t2,
                       trivial1, trivial2, num_devices=N_CORES)

    shared = dict(
        WmsgT1=np.ascontiguousarray(np.asarray(W_d2e, np.float32).T),
        WselfT1=np.ascontiguousarray(np.asarray(W_e_self, np.float32).T),
        WmsgT2=np.ascontiguousarray(np.asarray(W_e2d, np.float32).T),
        WselfT2=np.ascontiguousarray(np.asarray(W_d_self, np.float32).T),
        aff1=np.stack([np.asarray(b_e, np.float32), np.asarray(g_e, np.float32),
                       np.asarray(beta_e, np.float32)]),
        aff2=np.stack([np.asarray(b_d, np.float32), np.asarray(g_d, np.float32),
                       np.asarray(beta_d, np.float32)]),
        **prep,
    )
    in_maps = []
    for c in range(N_CORES):
        m = dict(shared)
        m["hD"] = np.ascontiguousarray(hD[c * Bc:(c + 1) * Bc])
        m["hE"] = np.ascontiguousarray(hE[c * Bc:(c + 1) * Bc])
        m["hDT"] = np.ascontiguousarray(m["hD"].transpose(0, 2, 1))
        m["hET"] = np.ascontiguousarray(m["hE"].transpose(0, 2, 1))
        in_maps.append(m)

    res = run_bass_kernel_spmd(nc, in_maps, list(range(N_CORES)))
    if res.exec_time_ns is not None:
        print(f"HW exec time: {res.exec_time_ns} ns", flush=True)
    hD_new = np.concatenate([res.results[c]["hD_new"] for c in range(N_CORES)], 0)
    hE_new = np.concatenate([res.results[c]["hE_new"] for c in range(N_CORES)], 0)
    return hD_new, hE_new


# revision 17
# speedup vs baseline: 1.3621x; 1.3621x over previous
"""Trainium2 Bass kernel for a bipartite GNN message-passing layer.

Strategy (8 NeuronCores, data-parallel over batch B=16 -> 2 batches/core):
  Per phase (D->E then E->D):
    1. transform-first: tX = hX @ W_msg^T computed per 128-row tile from
       host-pre-transposed activations (hXT input), written interleaved for
       both local batches to a DRAM scratch [N_src, 256] so one gather row
       serves both batches.
    2. edges are sorted by destination on the host and padded per dst tile
       to multiples of 128; dma_gather (2 SWDGE queues, multi-packet) pulls
       the per-edge source rows into SBUF.
    3. scatter-add realized as one-hot matmuls: O[e, dst] = (dl[e]==dst)
       built on-device in one tensor_tensor per dst tile (broadcast APs);
       PSUM accumulates agg[dst, 256] over the tile's edge chunks.
    4. epilogue per dst tile: agg*scale (ACT) + self-matmul PSUM add, relu,
       residual add, layernorm via sum/sumsq (ACT Square+accum) and
       rstd = exp(-0.5*ln(var+eps)) on ACT; normalize via ACT Identity
       with per-partition scale/bias. Writes the output and (phase 1) the
       next phase's transformed+interleaved scratch.

Engine budget notes (from NTFF profiling): avoid tensor_scalar with AP
scalars (~1us/op vs ~0.26us tensor_tensor), avoid DVE accum_out
(READ_ACCUMULATOR ~0.9us), avoid nc.vector.reciprocal (~1.9us), avoid two
PSUM operands in one DVE op (verifier rejects), dma_gather needs
single_packet=False above ~1k indices.
"""

import math
import os
import sys

import numpy as np

sys.path.insert(0, "/opt/trn_rl_repo")

import concourse.bacc as bacc
import concourse.bass as bass
import concourse.tile as tile
from concourse import mybir
from concourse.bass_utils import run_bass_kernel_spmd
from concourse.masks import make_identity

P = 128
H = 128
EPS = 1e-5
F32 = mybir.dt.float32
I32 = mybir.dt.int32
I16 = mybir.dt.int16
AF = mybir.ActivationFunctionType
OP = mybir.AluOpType

N_CORES = 8
MAXCH = 32  # max gather chunks (of 128 edges) per dma_gather group


# ----------------------------------------------------------------- host prep

def _prep_edges(edge_src, edge_dst, n_dst):
    """Sort edges by dst; per dst tile pad the edge list to a multiple of 128.
    Tiles with zero edges get zero chunks (skipped on device)."""
    order = np.argsort(edge_dst, kind="stable")
    src_s = edge_src[order]
    dst_s = edge_dst[order]
    n_tiles = n_dst // P
    bounds = np.searchsorted(dst_s, np.arange(n_tiles + 1) * P)
    g_src, g_dl, cpt = [], [], []
    for t in range(n_tiles):
        lo, hi = bounds[t], bounds[t + 1]
        n = hi - lo
        nch = (n + P - 1) // P
        pad = nch * P - n
        if n > 0:
            g_src.append(src_s[lo:hi])
            g_dl.append(dst_s[lo:hi] - t * P)
            if pad:
                g_src.append(np.zeros(pad, np.int64))
                g_dl.append(np.full(pad, -1, np.int64))
        cpt.append(nch)
    gather_src = np.concatenate(g_src).astype(np.int64) if g_src else np.zeros(0, np.int64)
    dl = np.concatenate(g_dl).astype(np.int32) if g_dl else np.zeros(0, np.int32)
    return gather_src, dl, np.array(cpt, np.int64)


def _wrap_idx16(idx):
    """[n] -> [128, n//16] int16, wrapped in 16 partitions, replicated x8."""
    n = len(idx)
    assert n % 16 == 0
    w = idx.astype(np.int16).reshape(n // 16, 16).T  # [16, n//16]
    return np.tile(w, (8, 1)).copy()  # [128, n//16]


def _col_mat(v, n_tiles):
    """[n_tiles*128] -> [128, n_tiles] where col t = v[t*128:(t+1)*128]."""
    return np.ascontiguousarray(v.reshape(n_tiles, P).T)


def _col_mat_chunks(dl):
    """padded dl [padE] -> [128, n_chunks] f32 (col c = chunk c's 128 values)."""
    if len(dl) == 0:
        return np.zeros((P, 1), np.float32)
    n_ch = len(dl) // P
    return np.ascontiguousarray(dl.reshape(n_ch, P).T.astype(np.float32))


def _make_groups(cpt):
    """Greedy-pack active dst tiles into gather groups of <= MAXCH chunks."""
    groups = []
    cur, cur_ch = [], 0
    for t, c in enumerate(cpt):
        if c == 0:
            continue
        if cur and cur_ch + c > MAXCH:
            groups.append((cur, cur_ch))
            cur, cur_ch = [], 0
        cur.append(t)
        cur_ch += int(c)
    if cur:
        groups.append((cur, cur_ch))
    return groups


# ------------------------------------------------------------ program build

class PhaseCfg:
    def __init__(self, name, n_src, n_dst, cpt, padE, use_sigmoid):
        self.name = name
        self.n_src = n_src
        self.n_dst = n_dst
        self.cpt = cpt
        self.padE = padE
        self.use_sigmoid = use_sigmoid
        self.groups = _make_groups(cpt)
        self.n_dst_tiles = n_dst // P
        self.n_chunks = int(cpt.sum())


def build_program(Bc, N_d, N_e, ph1, ph2, n_src_tiles_t2, trivial1, trivial2,
                  num_devices=N_CORES):
    """Emit the full SPMD Bass program."""
    nc = bacc.Bacc("TRN2", target_bir_lowering=False, debug=False,
                   enable_asserts=False, num_devices=num_devices,
                   num_swdge_queues=2, dynamic_dma_scratch_size=32768)
    BH = Bc * H

    # ---- DRAM tensors
    hD = nc.dram_tensor("hD", [Bc, N_d, H], F32, kind="ExternalInput").ap()
    hE = nc.dram_tensor("hE", [Bc, N_e, H], F32, kind="ExternalInput").ap()
    hDT = nc.dram_tensor("hDT", [Bc, H, N_d], F32, kind="ExternalInput").ap()
    hET = nc.dram_tensor("hET", [Bc, H, N_e], F32, kind="ExternalInput").ap()
    WmsgT1 = nc.dram_tensor("WmsgT1", [H, H], F32, kind="ExternalInput").ap()
    WselfT1 = nc.dram_tensor("WselfT1", [H, H], F32, kind="ExternalInput").ap()
    WmsgT2 = nc.dram_tensor("WmsgT2", [H, H], F32, kind="ExternalInput").ap()
    WselfT2 = nc.dram_tensor("WselfT2", [H, H], F32, kind="ExternalInput").ap()
    ew_mat = nc.dram_tensor("ew_mat", [P, N_e // P], F32, kind="ExternalInput").ap()
    cnt1_mat = nc.dram_tensor("cnt1_mat", [P, N_e // P], F32, kind="ExternalInput").ap()
    cnt2_mat = nc.dram_tensor("cnt2_mat", [P, N_d // P], F32, kind="ExternalInput").ap()
    idx1 = nc.dram_tensor("idx1", [P, ph1.padE // 16], I16, kind="ExternalInput").ap()
    dl1 = nc.dram_tensor("dl1", [P, max(ph1.n_chunks, 1)], F32, kind="ExternalInput").ap()
    idx2 = nc.dram_tensor("idx2", [P, ph2.padE // 16], I16, kind="ExternalInput").ap()
    dl2 = nc.dram_tensor("dl2", [P, max(ph2.n_chunks, 1)], F32, kind="ExternalInput").ap()
    aff1 = nc.dram_tensor("aff1", [3, H], F32, kind="ExternalInput").ap()
    aff2 = nc.dram_tensor("aff2", [3, H], F32, kind="ExternalInput").ap()

    hE_new = nc.dram_tensor("hE_new", [Bc, N_e, H], F32, kind="ExternalOutput").ap()
    hD_new = nc.dram_tensor("hD_new", [Bc, N_d, H], F32, kind="ExternalOutput").ap()

    tD = nc.dram_tensor("tD_scratch", [N_d, BH], F32, kind="Internal").ap()
    tE = nc.dram_tensor("tE_scratch", [n_src_tiles_t2 * P, BH], F32, kind="Internal").ap()

    with tile.TileContext(nc) as tc:
        import contextlib
        with contextlib.ExitStack() as ctx:
            # PSUM: every tile is padded to a full bank (8 banks total):
            # tr(2) + ps_agg(2) + ps_se(2) + ps_mm(2) = 8.
            const_tp = ctx.enter_context(tc.tile_pool(name="const", bufs=1))
            psum_tp = ctx.enter_context(tc.tile_pool(name="ps", bufs=2, space="PSUM"))
            tr_tp = ctx.enter_context(tc.tile_pool(name="tr", bufs=2, space="PSUM"))
            sb_tp = ctx.enter_context(tc.tile_pool(name="sb", bufs=3))
            g_tp = ctx.enter_context(tc.tile_pool(name="gath", bufs=3))

            ident = const_tp.tile([P, P], F32, tag="ident")
            make_identity(nc, ident[:])
            iota_i = const_tp.tile([P, P], I32, tag="iota_i")
            nc.gpsimd.iota(iota_i[:], pattern=[[1, P]], base=0, channel_multiplier=0)
            iota = const_tp.tile([P, P], F32, tag="iota")
            nc.vector.tensor_copy(iota[:], iota_i[:])
            # small per-partition constants
            eps_t = const_tp.tile([P, 1], F32, tag="eps")
            nc.gpsimd.memset(eps_t[:], EPS)
            invH_t = const_tp.tile([P, 1], F32, tag="invH")
            nc.gpsimd.memset(invH_t[:], 1.0 / H)
            neginvH_t = const_tp.tile([P, 1], F32, tag="neginvH")
            nc.gpsimd.memset(neginvH_t[:], -1.0 / H)
            neghalf_t = const_tp.tile([P, 1], F32, tag="neghalf")
            nc.gpsimd.memset(neghalf_t[:], -0.5)

            def load_const(ap_dram, shape, tag):
                t = const_tp.tile(shape, F32, tag=tag)
                nc.sync.dma_start(t[:], ap_dram)
                return t

            w_msg1 = load_const(WmsgT1[:, :], [P, H], "w0")
            w_self1 = load_const(WselfT1[:, :], [P, H], "w1")
            w_msg2 = load_const(WmsgT2[:, :], [P, H], "w2")
            w_self2 = load_const(WselfT2[:, :], [P, H], "w3")

            # scale1 = sigmoid(ew) / max(cnt1, 1);  scale2 = 1 / max(cnt2, 1)
            nt1, nt2 = N_e // P, N_d // P
            sc_a = const_tp.tile([P, nt1], F32, tag="sc_a")
            sc_b = const_tp.tile([P, nt1], F32, tag="sc_b")
            sc_bs = const_tp.tile([P, nt1], F32, tag="sc_bs")
            scale1 = const_tp.tile([P, nt1], F32, tag="scale1")
            nc.sync.dma_start(sc_a[:], ew_mat[:, :])
            nc.sync.dma_start(sc_b[:], cnt1_mat[:, :])
            nc.scalar.activation(sc_a[:], sc_a[:], AF.Sigmoid)
            nc.vector.tensor_scalar_max(sc_b[:], sc_b[:], 1.0)
            nc.vector.reciprocal_approx_accurate(sc_b[:], sc_b[:], scratch=sc_bs[:])
            nc.vector.tensor_tensor(out=scale1[:], in0=sc_a[:], in1=sc_b[:], op=OP.mult)
            sc_c = const_tp.tile([P, nt2], F32, tag="sc_c")
            sc_cs = const_tp.tile([P, nt2], F32, tag="sc_cs")
            scale2 = const_tp.tile([P, nt2], F32, tag="scale2")
            nc.sync.dma_start(sc_c[:], cnt2_mat[:, :])
            nc.vector.tensor_scalar_max(sc_c[:], sc_c[:], 1.0)
            nc.vector.reciprocal_approx_accurate(scale2[:], sc_c[:], scratch=sc_cs[:])

            # index / dst-local tables
            idx1_sb = const_tp.tile([P, ph1.padE // 16], I16, tag="idx1")
            nc.sync.dma_start(idx1_sb[:], idx1[:, :])
            dl1_sb = const_tp.tile([P, max(ph1.n_chunks, 1)], F32, tag="dl1")
            nc.sync.dma_start(dl1_sb[:], dl1[:, :])
            idx2_sb = const_tp.tile([P, ph2.padE // 16], I16, tag="idx2")
            nc.sync.dma_start(idx2_sb[:], idx2[:, :])
            dl2_sb = const_tp.tile([P, max(ph2.n_chunks, 1)], F32, tag="dl2")
            nc.sync.dma_start(dl2_sb[:], dl2[:, :])

            # affine params (bias matmul trick + broadcast g/beta)
            def affine_setup(aff_dram, trivial, tag):
                if trivial:
                    return None
                e1 = const_tp.tile([P, P], F32, tag=f"e1{tag}")
                nc.gpsimd.memset(e1[:], 0.0)
                nc.gpsimd.memset(e1[0:1, :], 1.0)
                bpad = const_tp.tile([P, BH], F32, tag=f"bp{tag}")
                nc.gpsimd.memset(bpad[:], 0.0)
                for b in range(Bc):
                    nc.sync.dma_start(bpad[0:1, b * H:(b + 1) * H], aff_dram[0:1, :])
                g_mat = const_tp.tile([P, H], F32, tag=f"g{tag}")
                beta_mat = const_tp.tile([P, H], F32, tag=f"bt{tag}")
                one_row = const_tp.tile([1, P], F32, tag=f"one{tag}")
                nc.gpsimd.memset(one_row[:], 1.0)
                gb_row = const_tp.tile([1, 2 * H], F32, tag=f"gbr{tag}")
                nc.sync.dma_start(gb_row[0:1, 0:H], aff_dram[1:2, :])
                nc.sync.dma_start(gb_row[0:1, H:2 * H], aff_dram[2:3, :])
                gb_ps = tr_tp.tile([P, 2 * H], F32, tag="tr")
                nc.tensor.matmul(out=gb_ps[:], lhsT=one_row[0:1, 0:P],
                                 rhs=gb_row[0:1, :], start=True, stop=True)
                nc.vector.tensor_copy(g_mat[:], gb_ps[:, 0:H])
                nc.vector.tensor_copy(beta_mat[:], gb_ps[:, H:2 * H])
                return e1, bpad, g_mat, beta_mat

            aff1_t = affine_setup(aff1, trivial1, "1")
            aff2_t = affine_setup(aff2, trivial2, "2")

            # ---------------- tD pass: tD = hD @ Wmsg1^T (from hDT, no transposes)
            for i in range(N_d // P):
                rows = slice(i * P, (i + 1) * P)
                hdT = sb_tp.tile([P, BH], F32, tag="td_hT")
                nc.sync.dma_start(hdT[:].rearrange("p (b r) -> p b r", b=Bc),
                                  hDT[0:Bc, :, rows].rearrange("b h r -> h b r"))
                ps_td = psum_tp.tile([P, BH], F32, tag="ps_mm")
                for b in range(Bc):
                    nc.tensor.matmul(out=ps_td[:, b * H:(b + 1) * H],
                                     lhsT=hdT[:, b * H:(b + 1) * H],
                                     rhs=w_msg1[:], start=True, stop=True)
                stage = sb_tp.tile([P, BH], F32, tag="td_stage")
                nc.scalar.activation(stage[:], ps_td[:], AF.Identity)
                nc.sync.dma_start(tD[rows, :], stage[:])

            # ---------------- phases
            def emit_phase(ph, t_dram, self_dram, selfT_dram, out_dram, w_self,
                           scale_mat, idx_sb, dl_sb, aff_t, next_cfg, qbase):
                chunk_base = {}
                off = 0
                for t in range(ph.n_dst_tiles):
                    chunk_base[t] = off
                    off += int(ph.cpt[t])

                group_edge_off = []
                off_e = 0
                for (tids, nch) in ph.groups:
                    group_edge_off.append(off_e)
                    off_e += nch * P

                group_of_tile = {}
                for gi, (tids, nch) in enumerate(ph.groups):
                    for t in tids:
                        group_of_tile[t] = gi

                agg_ready = {}

                def do_group(gi):
                    tids, nch = ph.groups[gi]
                    g_tile = g_tp.tile([P, MAXCH, BH], F32, tag="G")
                    n_idx = nch * P
                    off16 = group_edge_off[gi] // 16
                    nc.gpsimd.dma_gather(
                        out_ap=g_tile[:, 0:nch, :],
                        in_ap=t_dram[:, :],
                        idxs_ap=idx_sb[:, off16:off16 + n_idx // 16],
                        num_idxs=n_idx,
                        num_idxs_reg=n_idx,
                        elem_size=BH,
                        single_packet=False,
                        queue_num=(qbase + gi) % 2,
                    )
                    lc = 0
                    for t in tids:
                        nch_t = int(ph.cpt[t])
                        cc0 = chunk_base[t]
                        # one-hot for ALL chunks of this tile in one DVE op
                        oh = sb_tp.tile([P, 9, P], F32, tag="onehot")
                        nc.vector.tensor_tensor(
                            out=oh[:, 0:nch_t, :],
                            in0=iota[:, None, :].to_broadcast([P, nch_t, P]),
                            in1=dl_sb[:, cc0:cc0 + nch_t, None].to_broadcast([P, nch_t, P]),
                            op=OP.is_equal)
                        ps_agg = psum_tp.tile([P, BH], F32, tag="ps_agg")
                        for c in range(nch_t):
                            nc.tensor.matmul(out=ps_agg[:], lhsT=oh[:, c, :],
                                             rhs=g_tile[:, lc + c, :],
                                             start=(c == 0), stop=(c == nch_t - 1))
                        lc += nch_t
                        agg_ready[t] = ps_agg

                emitted = set()
                for t in range(ph.n_dst_tiles):
                    gi = group_of_tile.get(t)
                    if gi is not None and gi not in emitted:
                        do_group(gi)
                        emitted.add(gi)

                    rows = slice(t * P, (t + 1) * P)
                    h_self = sb_tp.tile([P, BH], F32, tag="h_self")
                    heT = sb_tp.tile([P, BH], F32, tag="ep_hT")
                    nc.sync.dma_start(
                        h_self[:].rearrange("p (b h) -> p b h", b=Bc),
                        self_dram[0:Bc, rows, :].rearrange("b r h -> r b h"))
                    nc.sync.dma_start(
                        heT[:].rearrange("p (b r) -> p b r", b=Bc),
                        selfT_dram[0:Bc, :, rows].rearrange("b h r -> h b r"))
                    ps_se = psum_tp.tile([P, BH], F32, tag="ps_se")
                    if aff_t is not None:
                        e1, bpad, g_mat, beta_mat = aff_t
                        nc.tensor.matmul(out=ps_se[:], lhsT=e1[:], rhs=bpad[:],
                                         start=True, stop=False, skip_group_check=True)
                    for b in range(Bc):
                        nc.tensor.matmul(out=ps_se[:, b * H:(b + 1) * H],
                                         lhsT=heT[:, b * H:(b + 1) * H],
                                         rhs=w_self[:],
                                         start=(aff_t is None), stop=True,
                                         skip_group_check=True)

                    # v = relu(agg*scale + sE)
                    v_t = sb_tp.tile([P, BH], F32, tag="v")
                    ps_agg = agg_ready.pop(t, None)
                    if ps_agg is not None:
                        agg_s = sb_tp.tile([P, BH], F32, tag="agg_s")
                        nc.scalar.activation(agg_s[:], ps_agg[:], AF.Identity,
                                             scale=scale_mat[:, t:t + 1])
                        u_t = sb_tp.tile([P, BH], F32, tag="u")
                        nc.vector.tensor_tensor(out=u_t[:], in0=agg_s[:],
                                                in1=ps_se[:], op=OP.add)
                        nc.scalar.activation(v_t[:], u_t[:], AF.Relu)
                    else:
                        nc.scalar.activation(v_t[:], ps_se[:], AF.Relu)

                    # x = h_self + v ; s1 = rowsum(x) ; s2 = rowsum(x^2)
                    x_t = sb_tp.tile([P, BH], F32, tag="x")
                    nc.vector.tensor_tensor(out=x_t[:], in0=h_self[:],
                                            in1=v_t[:], op=OP.add)
                    stats = sb_tp.tile([P, 12], F32, tag="stats")
                    s1 = stats[:, 0:Bc]
                    s2 = stats[:, 2:2 + Bc]
                    nc.vector.tensor_reduce(
                        out=s1, in_=x_t[:].rearrange("p (b h) -> p b h", b=Bc),
                        axis=mybir.AxisListType.X, op=OP.add)
                    junk = sb_tp.tile([P, BH], F32, tag="junk")
                    nc.vector.tensor_tensor(out=junk[:], in0=x_t[:], in1=x_t[:],
                                            op=OP.mult)
                    nc.vector.tensor_reduce(
                        out=s2, in_=junk[:].rearrange("p (b h) -> p b h", b=Bc),
                        axis=mybir.AxisListType.X, op=OP.add)
                    # tiny chain: VS = s2 - s1^2/H ; rstd = exp(-0.5*ln(VS/H+eps))
                    t1 = stats[:, 4:4 + Bc]
                    nc.vector.tensor_tensor(out=t1, in0=s1, in1=s1, op=OP.mult)
                    t2c = stats[:, 6:6 + Bc]
                    nc.vector.tensor_tensor(
                        out=t2c, in0=t1,
                        in1=invH_t[:, 0:1].to_broadcast([P, Bc]), op=OP.mult)
                    vs = stats[:, 8:8 + Bc]
                    nc.vector.tensor_tensor(out=vs, in0=s2, in1=t2c, op=OP.subtract)
                    lnv = sb_tp.tile([P, 6], F32, tag="lnv")
                    nc.scalar.activation(lnv[:, 0:Bc], vs, AF.Ln,
                                         scale=invH_t[:, 0:1], bias=eps_t[:, 0:1])
                    rstd = lnv[:, 2:2 + Bc]
                    nc.scalar.activation(rstd, lnv[:, 0:Bc], AF.Exp,
                                         scale=neghalf_t[:, 0:1])
                    negm = stats[:, 10:10 + Bc]
                    nc.vector.tensor_tensor(
                        out=negm, in0=s1,
                        in1=neginvH_t[:, 0:1].to_broadcast([P, Bc]), op=OP.mult)
                    negmr = lnv[:, 4:4 + Bc]
                    nc.vector.tensor_tensor(out=negmr, in0=negm, in1=rstd, op=OP.mult)

                    # y = x*rstd + negmr  [per batch; ACT Identity w/ scale+bias]
                    y_t = sb_tp.tile([P, BH], F32, tag="y")
                    for b in range(Bc):
                        bs = slice(b * H, (b + 1) * H)
                        nc.scalar.activation(y_t[:, bs], x_t[:, bs], AF.Identity,
                                             scale=rstd[:, b:b + 1],
                                             bias=negmr[:, b:b + 1])
                    if aff_t is not None:
                        e1, bpad, g_mat, beta_mat = aff_t
                        y3 = y_t[:].rearrange("p (b h) -> p b h", b=Bc)
                        nc.vector.tensor_tensor(
                            out=y3, in0=y3,
                            in1=g_mat[:, None, :].to_broadcast([P, Bc, H]), op=OP.mult)
                        nc.vector.tensor_tensor(
                            out=y3, in0=y3,
                            in1=beta_mat[:, None, :].to_broadcast([P, Bc, H]), op=OP.add)
                    nc.sync.dma_start(
                        out_dram[0:Bc, rows, :].rearrange("b r h -> r b h"),
                        y_t[:].rearrange("p (b h) -> p b h", b=Bc))

                    # fused next-phase transform: t2 = y @ Wmsg2^T
                    if next_cfg is not None:
                        w_next, t2_dram, n_src_tiles_next = next_cfg
                        if t < n_src_tiles_next:
                            ps_t2 = psum_tp.tile([P, BH], F32, tag="ps_mm")
                            for b in range(Bc):
                                ps_tr2 = tr_tp.tile([P, P], F32, tag="tr")
                                nc.tensor.transpose(out=ps_tr2[:],
                                                    in_=y_t[:, b * H:(b + 1) * H],
                                                    identity=ident[:])
                                yT = sb_tp.tile([P, P], F32, tag="t2_yT")
                                nc.scalar.activation(yT[:], ps_tr2[:], AF.Identity)
                                nc.tensor.matmul(out=ps_t2[:, b * H:(b + 1) * H],
                                                 lhsT=yT[:], rhs=w_next[:],
                                                 start=True, stop=True)
                            t2_stage = sb_tp.tile([P, BH], F32, tag="t2_stage")
                            nc.scalar.activation(t2_stage[:], ps_t2[:], AF.Identity)
                            nc.sync.dma_start(t2_dram[rows, :], t2_stage[:])

            emit_phase(ph1, tD, hE, hET, hE_new, w_self1, scale1, idx1_sb, dl1_sb,
                       aff1_t, (w_msg2, tE, n_src_tiles_t2), qbase=0)
            emit_phase(ph2, tE, hD, hDT, hD_new, w_self2, scale2, idx2_sb, dl2_sb,
                       aff2_t, None, qbase=1)

    nc.compile()
    return nc


# ----------------------------------------------------------------- kernel()

def _host_prep(hD, hE, edge_d2e, edge_e2d, error_weights):
    B, N_d, _ = hD.shape
    N_e = hE.shape[1]
    e1s, e1d = np.asarray(edge_d2e[0], np.int64), np.asarray(edge_d2e[1], np.int64)
    e2s, e2d = np.asarray(edge_e2d[0], np.int64), np.asarray(edge_e2d[1], np.int64)

    gs1, dl1, cpt1 = _prep_edges(e1s, e1d, N_e)
    gs2, dl2, cpt2 = _prep_edges(e2s, e2d, N_d)
    cnt1 = np.bincount(e1d, minlength=N_e).astype(np.float32)
    cnt2 = np.bincount(e2d, minlength=N_d).astype(np.float32)

    n_src_tiles_t2 = int(math.ceil((int(e2s.max()) + 1) / P)) if len(e2s) else 1

    prep = dict(
        idx1=_wrap_idx16(gs1), dl1=_col_mat_chunks(dl1),
        idx2=_wrap_idx16(gs2), dl2=_col_mat_chunks(dl2),
        ew_mat=_col_mat(np.asarray(error_weights, np.float32), N_e // P),
        cnt1_mat=_col_mat(cnt1, N_e // P),
        cnt2_mat=_col_mat(cnt2, N_d // P),
    )
    return prep, cpt1, cpt2, len(gs1), len(gs2), n_src_tiles_t2


def kernel(hD, hE, error_weights, W_d2e, W_e_self, b_e, g_e, beta_e,
           W_e2d, W_d_self, b_d, g_d, beta_d, edge_d2e, edge_e2d):
    hD = np.ascontiguousarray(np.asarray(hD, np.float32))
    hE = np.ascontiguousarray(np.asarray(hE, np.float32))
    B, N_d, _ = hD.shape
    N_e = hE.shape[1]
    assert B % N_CORES == 0
    Bc = B // N_CORES

    prep, cpt1, cpt2, padE1, padE2, n_src_tiles_t2 = _host_prep(
        hD, hE, edge_d2e, edge_e2d, error_weights)

    trivial1 = (np.all(np.asarray(b_e) == 0) and np.all(np.asarray(g_e) == 1)
                and np.all(np.asarray(beta_e) == 0))
    trivial2 = (np.all(np.asarray(b_d) == 0) and np.all(np.asarray(g_d) == 1)
                and np.all(np.asarray(beta_d) == 0))

    ph1 = PhaseCfg("d2e", N_d, N_e, cpt1, padE1, True)
    ph2 = PhaseCfg("e2d", n_src_tiles_t2 * P, N_d, cpt2, padE2, False)

    nc = build_program(Bc, N_d, N_e, ph1, ph2, n_src_tiles_t2,
                       trivial1, trivial2, num_devices=N_CORES)

    shared = dict(
        WmsgT1=np.ascontiguousarray(np.asarray(W_d2e, np.float32).T),
        WselfT1=np.ascontiguousarray(np.asarray(W_e_self, np.float32).T),
        WmsgT2=np.ascontiguousarray(np.asarray(W_e2d, np.float32).T),
        WselfT2=np.ascontiguousarray(np.asarray(W_d_self, np.float32).T),
        aff1=np.stack([np.asarray(b_e, np.float32), np.asarray(g_e, np.float32),
                       np.asarray(beta_e, np.float32)]),
        aff2=np.stack([np.asarray(b_d, np.float32), np.asarray(g_d, np.float32),
                       np.asarray(beta_d, np.float32)]),
        **prep,
    )
    in_maps = []
    for c in range(N_CORES):
        m = dict(shared)
        m["hD"] = np.ascontiguousarray(hD[c * Bc:(c + 1) * Bc])
        m["hE"] = np.ascontiguousarray(hE[c * Bc:(c + 1) * Bc])
        m["hDT"] = np.ascontiguousarray(m["hD"].transpose(0, 2, 1))
        m["hET"] = np.ascontiguousarray(m["hE"].transpose(0, 2, 1))
        in_maps.append(m)

    res = run_bass_kernel_spmd(nc, in_maps, list(range(N_CORES)))
    if res.exec_time_ns is not None:
        print(f"HW exec time: {res.exec_time_ns} ns", flush=True)
    hD_new = np.concatenate([res.results[c]["hD_new"] for c in range(N_CORES)], 0)
    hE_new = np.concatenate([res.results[c]["hE_new"] for c in range(N_CORES)], 0)
    return hD_new, hE_new


# revision 18
# speedup vs baseline: 1.4513x; 1.0655x over previous
"""Trainium2 Bass kernel for a bipartite GNN message-passing layer.

Strategy (8 NeuronCores, data-parallel over batch B=16 -> 2 batches/core):
  Per phase (D->E then E->D):
    1. transform-first: tX = hX @ W_msg^T computed per 128-row tile from
       host-pre-transposed activations (hXT input), written interleaved for
       both local batches to a DRAM scratch [N_src, 256] so one gather row
       serves both batches.
    2. edges are sorted by destination on the host and padded per dst tile
       to multiples of 128; dma_gather (2 SWDGE queues, multi-packet) pulls
       the per-edge source rows into SBUF.
    3. scatter-add realized as one-hot matmuls: O[e, dst] = (dl[e]==dst)
       built on-device in one tensor_tensor per dst tile (broadcast APs);
       PSUM accumulates agg[dst, 256] over the tile's edge chunks.
    4. epilogue per dst tile: agg*scale (ACT) + self-matmul PSUM add, relu,
       residual add, layernorm via sum/sumsq (ACT Square+accum) and
       rstd = exp(-0.5*ln(var+eps)) on ACT; normalize via ACT Identity
       with per-partition scale/bias. Writes the output and (phase 1) the
       next phase's transformed+interleaved scratch.

Engine budget notes (from NTFF profiling): avoid tensor_scalar with AP
scalars (~1us/op vs ~0.26us tensor_tensor), avoid DVE accum_out
(READ_ACCUMULATOR ~0.9us), avoid nc.vector.reciprocal (~1.9us), avoid two
PSUM operands in one DVE op (verifier rejects), dma_gather needs
single_packet=False above ~1k indices.
"""

import math
import os
import sys

import numpy as np

sys.path.insert(0, "/opt/trn_rl_repo")

import concourse.bacc as bacc
import concourse.bass as bass
import concourse.tile as tile
from concourse import mybir
from concourse.bass_utils import run_bass_kernel_spmd
from concourse.masks import make_identity

P = 128
H = 128
EPS = 1e-5
F32 = mybir.dt.float32
I32 = mybir.dt.int32
I16 = mybir.dt.int16
AF = mybir.ActivationFunctionType
OP = mybir.AluOpType

N_CORES = 8
MAXCH = 32  # max gather chunks (of 128 edges) per dma_gather group


# ----------------------------------------------------------------- host prep

def _prep_edges(edge_src, edge_dst, n_dst):
    """Sort edges by dst; per dst tile pad the edge list to a multiple of 128.
    Tiles with zero edges get zero chunks (skipped on device)."""
    order = np.argsort(edge_dst, kind="stable")
    src_s = edge_src[order]
    dst_s = edge_dst[order]
    n_tiles = n_dst // P
    bounds = np.searchsorted(dst_s, np.arange(n_tiles + 1) * P)
    g_src, g_dl, cpt = [], [], []
    for t in range(n_tiles):
        lo, hi = bounds[t], bounds[t + 1]
        n = hi - lo
        nch = (n + P - 1) // P
        pad = nch * P - n
        if n > 0:
            g_src.append(src_s[lo:hi])
            g_dl.append(dst_s[lo:hi] - t * P)
            if pad:
                g_src.append(np.zeros(pad, np.int64))
                g_dl.append(np.full(pad, -1, np.int64))
        cpt.append(nch)
    gather_src = np.concatenate(g_src).astype(np.int64) if g_src else np.zeros(0, np.int64)
    dl = np.concatenate(g_dl).astype(np.int32) if g_dl else np.zeros(0, np.int32)
    return gather_src, dl, np.array(cpt, np.int64)


def _wrap_idx16(idx):
    """[n] -> [128, n//16] int16, wrapped in 16 partitions, replicated x8."""
    n = len(idx)
    assert n % 16 == 0
    w = idx.astype(np.int16).reshape(n // 16, 16).T  # [16, n//16]
    return np.tile(w, (8, 1)).copy()  # [128, n//16]


def _col_mat(v, n_tiles):
    """[n_tiles*128] -> [128, n_tiles] where col t = v[t*128:(t+1)*128]."""
    return np.ascontiguousarray(v.reshape(n_tiles, P).T)


def _col_mat_chunks(dl):
    """padded dl [padE] -> [128, n_chunks] f32 (col c = chunk c's 128 values)."""
    if len(dl) == 0:
        return np.zeros((P, 1), np.float32)
    n_ch = len(dl) // P
    return np.ascontiguousarray(dl.reshape(n_ch, P).T.astype(np.float32))


def _make_groups(cpt):
    """Greedy-pack active dst tiles into gather groups of <= MAXCH chunks."""
    groups = []
    cur, cur_ch = [], 0
    for t, c in enumerate(cpt):
        if c == 0:
            continue
        if cur and cur_ch + c > MAXCH:
            groups.append((cur, cur_ch))
            cur, cur_ch = [], 0
        cur.append(t)
        cur_ch += int(c)
    if cur:
        groups.append((cur, cur_ch))
    return groups


# ------------------------------------------------------------ program build

class PhaseCfg:
    def __init__(self, name, n_src, n_dst, cpt, padE, use_sigmoid):
        self.name = name
        self.n_src = n_src
        self.n_dst = n_dst
        self.cpt = cpt
        self.padE = padE
        self.use_sigmoid = use_sigmoid
        self.groups = _make_groups(cpt)
        self.n_dst_tiles = n_dst // P
        self.n_chunks = int(cpt.sum())


def build_program(Bc, N_d, N_e, ph1, ph2, n_src_tiles_t2, trivial1, trivial2,
                  num_devices=N_CORES):
    """Emit the full SPMD Bass program."""
    nc = bacc.Bacc("TRN2", target_bir_lowering=False, debug=False,
                   enable_asserts=False, num_devices=num_devices,
                   num_swdge_queues=2, dynamic_dma_scratch_size=32768)
    BH = Bc * H

    # ---- DRAM tensors
    hD = nc.dram_tensor("hD", [Bc, N_d, H], F32, kind="ExternalInput").ap()
    hE = nc.dram_tensor("hE", [Bc, N_e, H], F32, kind="ExternalInput").ap()
    hDT = nc.dram_tensor("hDT", [Bc, H, N_d], F32, kind="ExternalInput").ap()
    hET = nc.dram_tensor("hET", [Bc, H, N_e], F32, kind="ExternalInput").ap()
    WmsgT1 = nc.dram_tensor("WmsgT1", [H, H], F32, kind="ExternalInput").ap()
    WselfT1 = nc.dram_tensor("WselfT1", [H, H], F32, kind="ExternalInput").ap()
    WmsgT2 = nc.dram_tensor("WmsgT2", [H, H], F32, kind="ExternalInput").ap()
    WselfT2 = nc.dram_tensor("WselfT2", [H, H], F32, kind="ExternalInput").ap()
    ew_mat = nc.dram_tensor("ew_mat", [P, N_e // P], F32, kind="ExternalInput").ap()
    cnt1_mat = nc.dram_tensor("cnt1_mat", [P, N_e // P], F32, kind="ExternalInput").ap()
    cnt2_mat = nc.dram_tensor("cnt2_mat", [P, N_d // P], F32, kind="ExternalInput").ap()
    idx1 = nc.dram_tensor("idx1", [P, ph1.padE // 16], I16, kind="ExternalInput").ap()
    dl1 = nc.dram_tensor("dl1", [P, max(ph1.n_chunks, 1)], F32, kind="ExternalInput").ap()
    idx2 = nc.dram_tensor("idx2", [P, ph2.padE // 16], I16, kind="ExternalInput").ap()
    dl2 = nc.dram_tensor("dl2", [P, max(ph2.n_chunks, 1)], F32, kind="ExternalInput").ap()
    aff1 = nc.dram_tensor("aff1", [3, H], F32, kind="ExternalInput").ap()
    aff2 = nc.dram_tensor("aff2", [3, H], F32, kind="ExternalInput").ap()

    hE_new = nc.dram_tensor("hE_new", [Bc, N_e, H], F32, kind="ExternalOutput").ap()
    hD_new = nc.dram_tensor("hD_new", [Bc, N_d, H], F32, kind="ExternalOutput").ap()

    tD = nc.dram_tensor("tD_scratch", [N_d, BH], F32, kind="Internal").ap()
    tE = nc.dram_tensor("tE_scratch", [n_src_tiles_t2 * P, BH], F32, kind="Internal").ap()

    with tile.TileContext(nc) as tc:
        import contextlib
        with contextlib.ExitStack() as ctx:
            # PSUM: every tile is padded to a full bank (8 banks total):
            # tr(2) + ps_agg(2) + ps_se(2) + ps_mm(2) = 8.
            const_tp = ctx.enter_context(tc.tile_pool(name="const", bufs=1))
            psum_tp = ctx.enter_context(tc.tile_pool(name="ps", bufs=2, space="PSUM"))
            tr_tp = ctx.enter_context(tc.tile_pool(name="tr", bufs=2, space="PSUM"))
            sb_tp = ctx.enter_context(tc.tile_pool(name="sb", bufs=3))
            x_tp = ctx.enter_context(tc.tile_pool(name="xp", bufs=10))
            g_tp = ctx.enter_context(tc.tile_pool(name="gath", bufs=3))

            ident = const_tp.tile([P, P], F32, tag="ident")
            make_identity(nc, ident[:])
            iota_i = const_tp.tile([P, P], I32, tag="iota_i")
            nc.gpsimd.iota(iota_i[:], pattern=[[1, P]], base=0, channel_multiplier=0)
            iota = const_tp.tile([P, P], F32, tag="iota")
            nc.vector.tensor_copy(iota[:], iota_i[:])
            # small per-partition constants
            eps_t = const_tp.tile([P, 1], F32, tag="eps")
            nc.gpsimd.memset(eps_t[:], EPS)
            invH_t = const_tp.tile([P, 1], F32, tag="invH")
            nc.gpsimd.memset(invH_t[:], 1.0 / H)
            neginvH_t = const_tp.tile([P, 1], F32, tag="neginvH")
            nc.gpsimd.memset(neginvH_t[:], -1.0 / H)
            neghalf_t = const_tp.tile([P, 1], F32, tag="neghalf")
            nc.gpsimd.memset(neghalf_t[:], -0.5)

            def load_const(ap_dram, shape, tag):
                t = const_tp.tile(shape, F32, tag=tag)
                nc.sync.dma_start(t[:], ap_dram)
                return t

            w_msg1 = load_const(WmsgT1[:, :], [P, H], "w0")
            w_self1 = load_const(WselfT1[:, :], [P, H], "w1")
            w_msg2 = load_const(WmsgT2[:, :], [P, H], "w2")
            w_self2 = load_const(WselfT2[:, :], [P, H], "w3")

            # scale1 = sigmoid(ew) / max(cnt1, 1);  scale2 = 1 / max(cnt2, 1)
            nt1, nt2 = N_e // P, N_d // P
            sc_a = const_tp.tile([P, nt1], F32, tag="sc_a")
            sc_b = const_tp.tile([P, nt1], F32, tag="sc_b")
            sc_bs = const_tp.tile([P, nt1], F32, tag="sc_bs")
            scale1 = const_tp.tile([P, nt1], F32, tag="scale1")
            nc.sync.dma_start(sc_a[:], ew_mat[:, :])
            nc.sync.dma_start(sc_b[:], cnt1_mat[:, :])
            nc.scalar.activation(sc_a[:], sc_a[:], AF.Sigmoid)
            nc.vector.tensor_scalar_max(sc_b[:], sc_b[:], 1.0)
            nc.vector.reciprocal_approx_accurate(sc_b[:], sc_b[:], scratch=sc_bs[:])
            nc.vector.tensor_tensor(out=scale1[:], in0=sc_a[:], in1=sc_b[:], op=OP.mult)
            sc_c = const_tp.tile([P, nt2], F32, tag="sc_c")
            sc_cs = const_tp.tile([P, nt2], F32, tag="sc_cs")
            scale2 = const_tp.tile([P, nt2], F32, tag="scale2")
            nc.sync.dma_start(sc_c[:], cnt2_mat[:, :])
            nc.vector.tensor_scalar_max(sc_c[:], sc_c[:], 1.0)
            nc.vector.reciprocal_approx_accurate(scale2[:], sc_c[:], scratch=sc_cs[:])

            # index / dst-local tables
            idx1_sb = const_tp.tile([P, ph1.padE // 16], I16, tag="idx1")
            nc.sync.dma_start(idx1_sb[:], idx1[:, :])
            dl1_sb = const_tp.tile([P, max(ph1.n_chunks, 1)], F32, tag="dl1")
            nc.sync.dma_start(dl1_sb[:], dl1[:, :])
            idx2_sb = const_tp.tile([P, ph2.padE // 16], I16, tag="idx2")
            nc.sync.dma_start(idx2_sb[:], idx2[:, :])
            dl2_sb = const_tp.tile([P, max(ph2.n_chunks, 1)], F32, tag="dl2")
            nc.sync.dma_start(dl2_sb[:], dl2[:, :])

            # affine params (bias matmul trick + broadcast g/beta)
            def affine_setup(aff_dram, trivial, tag):
                if trivial:
                    return None
                e1 = const_tp.tile([P, P], F32, tag=f"e1{tag}")
                nc.gpsimd.memset(e1[:], 0.0)
                nc.gpsimd.memset(e1[0:1, :], 1.0)
                bpad = const_tp.tile([P, BH], F32, tag=f"bp{tag}")
                nc.gpsimd.memset(bpad[:], 0.0)
                for b in range(Bc):
                    nc.sync.dma_start(bpad[0:1, b * H:(b + 1) * H], aff_dram[0:1, :])
                g_mat = const_tp.tile([P, H], F32, tag=f"g{tag}")
                beta_mat = const_tp.tile([P, H], F32, tag=f"bt{tag}")
                one_row = const_tp.tile([1, P], F32, tag=f"one{tag}")
                nc.gpsimd.memset(one_row[:], 1.0)
                gb_row = const_tp.tile([1, 2 * H], F32, tag=f"gbr{tag}")
                nc.sync.dma_start(gb_row[0:1, 0:H], aff_dram[1:2, :])
                nc.sync.dma_start(gb_row[0:1, H:2 * H], aff_dram[2:3, :])
                gb_ps = tr_tp.tile([P, 2 * H], F32, tag="tr")
                nc.tensor.matmul(out=gb_ps[:], lhsT=one_row[0:1, 0:P],
                                 rhs=gb_row[0:1, :], start=True, stop=True)
                nc.vector.tensor_copy(g_mat[:], gb_ps[:, 0:H])
                nc.vector.tensor_copy(beta_mat[:], gb_ps[:, H:2 * H])
                return e1, bpad, g_mat, beta_mat

            aff1_t = affine_setup(aff1, trivial1, "1")
            aff2_t = affine_setup(aff2, trivial2, "2")

            # ---------------- tD pass: tD = hD @ Wmsg1^T (from hDT, no transposes)
            for i in range(N_d // P):
                rows = slice(i * P, (i + 1) * P)
                hdT = sb_tp.tile([P, BH], F32, tag="td_hT")
                nc.sync.dma_start(hdT[:].rearrange("p (b r) -> p b r", b=Bc),
                                  hDT[0:Bc, :, rows].rearrange("b h r -> h b r"))
                ps_td = psum_tp.tile([P, BH], F32, tag="ps_mm")
                for b in range(Bc):
                    nc.tensor.matmul(out=ps_td[:, b * H:(b + 1) * H],
                                     lhsT=hdT[:, b * H:(b + 1) * H],
                                     rhs=w_msg1[:], start=True, stop=True)
                stage = sb_tp.tile([P, BH], F32, tag="td_stage")
                nc.scalar.activation(stage[:], ps_td[:], AF.Identity)
                nc.sync.dma_start(tD[rows, :], stage[:])

            # ---------------- phases
            def emit_phase(ph, t_dram, self_dram, selfT_dram, out_dram, w_self,
                           scale_mat, idx_sb, dl_sb, aff_t, next_cfg, qbase):
                chunk_base = {}
                off = 0
                for t in range(ph.n_dst_tiles):
                    chunk_base[t] = off
                    off += int(ph.cpt[t])

                group_edge_off = []
                off_e = 0
                for (tids, nch) in ph.groups:
                    group_edge_off.append(off_e)
                    off_e += nch * P

                group_of_tile = {}
                for gi, (tids, nch) in enumerate(ph.groups):
                    for t in tids:
                        group_of_tile[t] = gi

                agg_ready = {}

                def do_group(gi):
                    tids, nch = ph.groups[gi]
                    g_tile = g_tp.tile([P, MAXCH, BH], F32, tag="G")
                    n_idx = nch * P
                    off16 = group_edge_off[gi] // 16
                    nc.gpsimd.dma_gather(
                        out_ap=g_tile[:, 0:nch, :],
                        in_ap=t_dram[:, :],
                        idxs_ap=idx_sb[:, off16:off16 + n_idx // 16],
                        num_idxs=n_idx,
                        num_idxs_reg=n_idx,
                        elem_size=BH,
                        single_packet=False,
                        queue_num=(qbase + gi) % 2,
                    )
                    lc = 0
                    for t in tids:
                        nch_t = int(ph.cpt[t])
                        cc0 = chunk_base[t]
                        # one-hot for ALL chunks of this tile in one DVE op
                        oh = sb_tp.tile([P, 9, P], F32, tag="onehot")
                        nc.vector.tensor_tensor(
                            out=oh[:, 0:nch_t, :],
                            in0=iota[:, None, :].to_broadcast([P, nch_t, P]),
                            in1=dl_sb[:, cc0:cc0 + nch_t, None].to_broadcast([P, nch_t, P]),
                            op=OP.is_equal)
                        ps_agg = psum_tp.tile([P, BH], F32, tag="ps_agg")
                        for c in range(nch_t):
                            nc.tensor.matmul(out=ps_agg[:], lhsT=oh[:, c, :],
                                             rhs=g_tile[:, lc + c, :],
                                             start=(c == 0), stop=(c == nch_t - 1))
                        lc += nch_t
                        agg_ready[t] = ps_agg

                emitted = set()
                WIN = 8

                def stage1(t, wstats, wslot):
                    gi = group_of_tile.get(t)
                    if gi is not None and gi not in emitted:
                        do_group(gi)
                        emitted.add(gi)
                    rows = slice(t * P, (t + 1) * P)
                    h_self = sb_tp.tile([P, BH], F32, tag="h_self")
                    heT = sb_tp.tile([P, BH], F32, tag="ep_hT")
                    nc.sync.dma_start(
                        h_self[:].rearrange("p (b h) -> p b h", b=Bc),
                        self_dram[0:Bc, rows, :].rearrange("b r h -> r b h"))
                    nc.sync.dma_start(
                        heT[:].rearrange("p (b r) -> p b r", b=Bc),
                        selfT_dram[0:Bc, :, rows].rearrange("b h r -> h b r"))
                    ps_se = psum_tp.tile([P, BH], F32, tag="ps_se")
                    if aff_t is not None:
                        e1, bpad, g_mat, beta_mat = aff_t
                        nc.tensor.matmul(out=ps_se[:], lhsT=e1[:], rhs=bpad[:],
                                         start=True, stop=False, skip_group_check=True)
                    for b in range(Bc):
                        nc.tensor.matmul(out=ps_se[:, b * H:(b + 1) * H],
                                         lhsT=heT[:, b * H:(b + 1) * H],
                                         rhs=w_self[:],
                                         start=(aff_t is None), stop=True,
                                         skip_group_check=True)
                    v_t = sb_tp.tile([P, BH], F32, tag="v")
                    ps_agg = agg_ready.pop(t, None)
                    if ps_agg is not None:
                        agg_s = sb_tp.tile([P, BH], F32, tag="agg_s")
                        nc.scalar.activation(agg_s[:], ps_agg[:], AF.Identity,
                                             scale=scale_mat[:, t:t + 1])
                        u_t = sb_tp.tile([P, BH], F32, tag="u")
                        nc.vector.tensor_tensor(out=u_t[:], in0=agg_s[:],
                                                in1=ps_se[:], op=OP.add)
                        nc.scalar.activation(v_t[:], u_t[:], AF.Relu)
                    else:
                        nc.scalar.activation(v_t[:], ps_se[:], AF.Relu)
                    x_t = x_tp.tile([P, BH], F32, tag="x")
                    nc.vector.tensor_tensor(out=x_t[:], in0=h_self[:],
                                            in1=v_t[:], op=OP.add)
                    sl = slice(2 * wslot, 2 * wslot + Bc)
                    nc.vector.tensor_reduce(
                        out=wstats[:, 0:16][:, sl],
                        in_=x_t[:].rearrange("p (b h) -> p b h", b=Bc),
                        axis=mybir.AxisListType.X, op=OP.add)
                    junk = sb_tp.tile([P, BH], F32, tag="junk")
                    nc.vector.tensor_tensor(out=junk[:], in0=x_t[:], in1=x_t[:],
                                            op=OP.mult)
                    nc.vector.tensor_reduce(
                        out=wstats[:, 16:32][:, sl],
                        in_=junk[:].rearrange("p (b h) -> p b h", b=Bc),
                        axis=mybir.AxisListType.X, op=OP.add)
                    return x_t

                def window_math(wstats, nw):
                    n = 2 * nw
                    s1w, s2w = wstats[:, 0:n], wstats[:, 16:16 + n]
                    t1 = wstats[:, 32:32 + n]
                    nc.vector.tensor_tensor(out=t1, in0=s1w, in1=s1w, op=OP.mult)
                    nc.vector.tensor_tensor(
                        out=t1, in0=t1,
                        in1=invH_t[:, 0:1].to_broadcast([P, n]), op=OP.mult)
                    vsw = wstats[:, 48:48 + n]
                    nc.vector.tensor_tensor(out=vsw, in0=s2w, in1=t1, op=OP.subtract)
                    lnw = wstats[:, 64:64 + n]
                    nc.scalar.activation(lnw, vsw, AF.Ln,
                                         scale=invH_t[:, 0:1], bias=eps_t[:, 0:1])
                    rstdw = wstats[:, 80:80 + n]
                    nc.scalar.activation(rstdw, lnw, AF.Exp,
                                         scale=neghalf_t[:, 0:1])
                    negmw = wstats[:, 96:96 + n]
                    nc.vector.tensor_tensor(
                        out=negmw, in0=s1w,
                        in1=neginvH_t[:, 0:1].to_broadcast([P, n]), op=OP.mult)
                    negmrw = wstats[:, 112:112 + n]
                    nc.vector.tensor_tensor(out=negmrw, in0=negmw, in1=rstdw,
                                            op=OP.mult)
                    return rstdw, negmrw

                def stage2(t, x_t, wstats, wslot):
                    rows = slice(t * P, (t + 1) * P)
                    rstdw = wstats[:, 80:96]
                    negmrw = wstats[:, 112:128]
                    y_t = sb_tp.tile([P, BH], F32, tag="y")
                    for b in range(Bc):
                        bs = slice(b * H, (b + 1) * H)
                        c = 2 * wslot + b
                        nc.scalar.activation(y_t[:, bs], x_t[:, bs], AF.Identity,
                                             scale=rstdw[:, c:c + 1],
                                             bias=negmrw[:, c:c + 1])
                    if aff_t is not None:
                        e1, bpad, g_mat, beta_mat = aff_t
                        y3 = y_t[:].rearrange("p (b h) -> p b h", b=Bc)
                        nc.vector.tensor_tensor(
                            out=y3, in0=y3,
                            in1=g_mat[:, None, :].to_broadcast([P, Bc, H]), op=OP.mult)
                        nc.vector.tensor_tensor(
                            out=y3, in0=y3,
                            in1=beta_mat[:, None, :].to_broadcast([P, Bc, H]), op=OP.add)
                    nc.sync.dma_start(
                        out_dram[0:Bc, rows, :].rearrange("b r h -> r b h"),
                        y_t[:].rearrange("p (b h) -> p b h", b=Bc))
                    if next_cfg is not None:
                        w_next, t2_dram, n_src_tiles_next = next_cfg
                        if t < n_src_tiles_next:
                            ps_t2 = psum_tp.tile([P, BH], F32, tag="ps_mm")
                            for b in range(Bc):
                                ps_tr2 = tr_tp.tile([P, P], F32, tag="tr")
                                nc.tensor.transpose(out=ps_tr2[:],
                                                    in_=y_t[:, b * H:(b + 1) * H],
                                                    identity=ident[:])
                                yT = sb_tp.tile([P, P], F32, tag="t2_yT")
                                nc.scalar.activation(yT[:], ps_tr2[:], AF.Identity)
                                nc.tensor.matmul(out=ps_t2[:, b * H:(b + 1) * H],
                                                 lhsT=yT[:], rhs=w_next[:],
                                                 start=True, stop=True)
                            t2_stage = sb_tp.tile([P, BH], F32, tag="t2_stage")
                            nc.scalar.activation(t2_stage[:], ps_t2[:], AF.Identity)
                            nc.sync.dma_start(t2_dram[rows, :], t2_stage[:])

                all_tiles = list(range(ph.n_dst_tiles))
                for w0 in range(0, len(all_tiles), WIN):
                    wtiles = all_tiles[w0:w0 + WIN]
                    wstats = sb_tp.tile([P, 128], F32, tag="wstats")
                    xs = []
                    for i, t in enumerate(wtiles):
                        xs.append(stage1(t, wstats, i))
                    window_math(wstats, len(wtiles))
                    for i, t in enumerate(wtiles):
                        stage2(t, xs[i], wstats, i)

            emit_phase(ph1, tD, hE, hET, hE_new, w_self1, scale1, idx1_sb, dl1_sb,
                       aff1_t, (w_msg2, tE, n_src_tiles_t2), qbase=0)
            emit_phase(ph2, tE, hD, hDT, hD_new, w_self2, scale2, idx2_sb, dl2_sb,
                       aff2_t, None, qbase=1)

    nc.compile()
    return nc


# ----------------------------------------------------------------- kernel()

def _host_prep(hD, hE, edge_d2e, edge_e2d, error_weights):
    B, N_d, _ = hD.shape
    N_e = hE.shape[1]
    e1s, e1d = np.asarray(edge_d2e[0], np.int64), np.asarray(edge_d2e[1], np.int64)
    e2s, e2d = np.asarray(edge_e2d[0], np.int64), np.asarray(edge_e2d[1], np.int64)

    gs1, dl1, cpt1 = _prep_edges(e1s, e1d, N_e)
    gs2, dl2, cpt2 = _prep_edges(e2s, e2d, N_d)
    cnt1 = np.bincount(e1d, minlength=N_e).astype(np.float32)
    cnt2 = np.bincount(e2d, minlength=N_d).astype(np.float32)

    n_src_tiles_t2 = int(math.ceil((int(e2s.max()) + 1) / P)) if len(e2s) else 1

    prep = dict(
        idx1=_wrap_idx16(gs1), dl1=_col_mat_chunks(dl1),
        idx2=_wrap_idx16(gs2), dl2=_col_mat_chunks(dl2),
        ew_mat=_col_mat(np.asarray(error_weights, np.float32), N_e // P),
        cnt1_mat=_col_mat(cnt1, N_e // P),
        cnt2_mat=_col_mat(cnt2, N_d // P),
    )
    return prep, cpt1, cpt2, len(gs1), len(gs2), n_src_tiles_t2


def kernel(hD, hE, error_weights, W_d2e, W_e_self, b_e, g_e, beta_e,
           W_e2d, W_d_self, b_d, g_d, beta_d, edge_d2e, edge_e2d):
    hD = np.ascontiguousarray(np.asarray(hD, np.float32))
    hE = np.ascontiguousarray(np.asarray(hE, np.float32))
    B, N_d, _ = hD.shape
    N_e = hE.shape[1]
    assert B % N_CORES == 0
    Bc = B // N_CORES

    prep, cpt1, cpt2, padE1, padE2, n_src_tiles_t2 = _host_prep(
        hD, hE, edge_d2e, edge_e2d, error_weights)

    trivial1 = (np.all(np.asarray(b_e) == 0) and np.all(np.asarray(g_e) == 1)
                and np.all(np.asarray(beta_e) == 0))
    trivial2 = (np.all(np.asarray(b_d) == 0) and np.all(np.asarray(g_d) == 1)
                and np.all(np.asarray(beta_d) == 0))

    ph1 = PhaseCfg("d2e", N_d, N_e, cpt1, padE1, True)
    ph2 = PhaseCfg("e2d", n_src_tiles_t2 * P, N_d, cpt2, padE2, False)

    nc = build_program(Bc, N_d, N_e, ph1, ph2, n_src_tiles_t2,
                       trivial1, trivial2, num_devices=N_CORES)

    shared = dict(
        WmsgT1=np.ascontiguousarray(np.asarray(W_d2e, np.float32).T),
        WselfT1=np.ascontiguousarray(np.asarray(W_e_self, np.float32).T),
        WmsgT2=np.ascontiguousarray(np.asarray(W_e2d, np.float32).T),
        WselfT2=np.ascontiguousarray(np.asarray(W_d_self, np.float32).T),
        aff1=np.stack([np.asarray(b_e, np.float32), np.asarray(g_e, np.float32),
                       np.asarray(beta_e, np.float32)]),
        aff2=np.stack([np.asarray(b_d, np.float32), np.asarray(g_d, np.float32),
                       np.asarray(beta_d, np.float32)]),
        **prep,
    )
    in_maps = []
    for c in range(N_CORES):
        m = dict(shared)
        m["hD"] = np.ascontiguousarray(hD[c * Bc:(c + 1) * Bc])
        m["hE"] = np.ascontiguousarray(hE[c * Bc:(c + 1) * Bc])
        m["hDT"] = np.ascontiguousarray(m["hD"].transpose(0, 2, 1))
        m["hET"] = np.ascontiguousarray(m["hE"].transpose(0, 2, 1))
        in_maps.append(m)

    res = run_bass_kernel_spmd(nc, in_maps, list(range(N_CORES)))
    if res.exec_time_ns is not None:
        print(f"HW exec time: {res.exec_time_ns} ns", flush=True)
    hD_new = np.concatenate([res.results[c]["hD_new"] for c in range(N_CORES)], 0)
    hE_new = np.concatenate([res.results[c]["hE_new"] for c in range(N_CORES)], 0)
    return hD_new, hE_new
